# revision 6
# baseline (speedup 1.0000x reference)
"""KMeans assignment kernel for Trainium2 (8 NeuronCores, SPMD).

argmin_k ||f_n - c_k||^2  ==  argmax_k (2*f.c_k - ||c_k||^2)   (x_sq drop is
order-preserving).  Cross products run on the PE array in float32r
(12-mantissa-bit fp32) with a 3-pass hi/lo split:
    f@c ~= hi_f@hi_c + hi_f@lo_c + lo_f@hi_c        (abs err ~2e-5, fp32-grade)
at 1 cycle/row instead of fp32's 4.  The -|c|^2 bias enters PSUM as two
rank-1 f32r matmuls (ones x ncsq_hi/lo).  Row-wise argmax via DVE
max/max_index directly on PSUM.

Sharding: features split over N across 8 cores (data parallel); centroids
replicated; no cross-core communication.
"""
import sys

sys.path.insert(0, "/opt/trn_rl_repo")

import numpy as np
from contextlib import ExitStack

import concourse.bacc as bacc
import concourse.mybir as mybir
from concourse import tile
from concourse.bass_utils import run_bass_kernel_spmd
from concourse.masks import make_identity

N, D, K = 131072, 512, 1024
N_CORES = 8
N_PER_CORE = N // N_CORES          # 16384
N_TILES = N_PER_CORE // 128        # 128 row-tiles per core
ND = D // 128                      # 4 contraction tiles
F32 = mybir.dt.float32
F32R = mybir.dt.float32r
U32 = mybir.dt.uint32

_cached = {}


def build_bass(n_tiles: int = N_TILES):
    n_rows = n_tiles * 128
    nc = bacc.Bacc()
    feat = nc.declare_dram_parameter("features", [n_rows, D], F32, isOutput=False)
    cent = nc.declare_dram_parameter("centroids", [D, K], F32, isOutput=False)
    ncsq = nc.declare_dram_parameter("ncsq", [1, K], F32, isOutput=False)
    out = nc.declare_dram_parameter("out", [n_rows, 1], F32, isOutput=True)

    with tile.TileContext(nc) as tc, ExitStack() as ctx:
        const = ctx.enter_context(tc.tile_pool(name="const", bufs=1))
        work = ctx.enter_context(tc.tile_pool(name="work", bufs=3))
        red = ctx.enter_context(tc.tile_pool(name="red", bufs=4))
        psA = ctx.enter_context(tc.tile_pool(name="psA", bufs=2, space="PSUM"))
        psB = ctx.enter_context(tc.tile_pool(name="psB", bufs=2, space="PSUM"))

        ident = const.tile([128, 128], F32)
        make_identity(nc, ident[:])

        # centroids resident in SBUF, split hi/lo f32r; layout [128, ND*K]
        ctile = const.tile([128, ND * K], F32)
        nc.sync.dma_start(
            out=ctile[:].rearrange("p (a k) -> p a k", a=ND),
            in_=cent[:].rearrange("(a p) k -> p a k", p=128),
        )
        c_hi = const.tile([128, ND * K], F32R)
        c_lo = const.tile([128, ND * K], F32R)
        nc.vector.tensor_copy(out=c_hi[:], in_=ctile[:])
        nc.vector.tensor_tensor(out=c_lo[:], in0=ctile[:], in1=c_hi[:].bitcast(F32),
                                op=mybir.AluOpType.subtract)

        # -|c|^2 bias row, split hi/lo; plus a ones row for rank-1 matmuls
        ncsq_t = const.tile([1, K], F32)
        nc.sync.dma_start(out=ncsq_t[:], in_=ncsq[:])
        ncsq_hi = const.tile([1, K], F32R)
        ncsq_lo = const.tile([1, K], F32R)
        nc.vector.tensor_copy(out=ncsq_hi[:], in_=ncsq_t[:])
        nc.vector.tensor_tensor(out=ncsq_lo[:], in0=ncsq_t[:],
                                in1=ncsq_hi[:].bitcast(F32),
                                op=mybir.AluOpType.subtract)
        ones_f = const.tile([1, 128], F32)
        nc.vector.memset(ones_f[:], 1.0)
        ones_t = const.tile([1, 128], F32R)
        nc.vector.tensor_copy(out=ones_t[:], in_=ones_f[:])

        # per-row argmax indices accumulate here ([p, t*8] layout), cast at end
        idx8 = const.tile([128, n_tiles * 8], U32)
        fbuf = const.tile([128, n_tiles], F32)

        for rt in range(n_tiles):
            ftile = work.tile([128, D], F32, tag="ftile")
            nc.sync.dma_start(out=ftile[:], in_=feat[rt * 128:(rt + 1) * 128, :])

            # transpose features tile -> [D, rows] chunks (exact fp32)
            tp = psA.tile([128, ND * 128], F32, tag="tp")
            for d in range(ND):
                nc.tensor.transpose(tp[:, d * 128:(d + 1) * 128],
                                    ftile[:, d * 128:(d + 1) * 128], ident[:])
            ftT = work.tile([128, D], F32, tag="ftT")
            nc.scalar.copy(out=ftT[:], in_=tp[:])

            # hi/lo split (round-to-nearest-12-bit + exact residual)
            f_hi = work.tile([128, D], F32R, tag="f_hi")
            f_lo = work.tile([128, D], F32R, tag="f_lo")
            nc.vector.tensor_copy(out=f_hi[:], in_=ftT[:])
            nc.vector.tensor_tensor(out=f_lo[:], in0=ftT[:],
                                    in1=f_hi[:].bitcast(F32),
                                    op=mybir.AluOpType.subtract)

            # m = 2*cross - |c|^2 accumulated in PSUM [128, K]
            mp = psB.tile([128, K], F32, tag="mp")
            for kh in range(2):
                ks = slice(kh * 512, (kh + 1) * 512)
                mslc = mp[:, ks]
                first = True
                for fa, ca in ((f_hi, c_hi), (f_hi, c_lo), (f_lo, c_hi)):
                    for d in range(ND):
                        nc.tensor.matmul(
                            mslc,
                            lhsT=fa[:, d * 128:(d + 1) * 128],
                            rhs=ca[:, d * K + kh * 512:d * K + (kh + 1) * 512],
                            start=first, stop=False)
                        first = False
                nc.tensor.matmul(mslc, lhsT=ones_t[:], rhs=ncsq_hi[:, ks],
                                 start=False, stop=False)
                nc.tensor.matmul(mslc, lhsT=ones_t[:], rhs=ncsq_lo[:, ks],
                                 start=False, stop=True)

            mv = red.tile([128, 8], F32, tag="mv")
            nc.vector.max(mv[:], mp[:])
            nc.vector.max_index(idx8[:, rt * 8:(rt + 1) * 8], mv[:], mp[:])

        # gather col 0 of each 8-block, cast u32 -> f32, store
        nc.vector.tensor_copy(out=fbuf[:], in_=idx8[:, 0:n_tiles * 8:8])
        nc.sync.dma_start(out=out[:, 0].rearrange("(t p) -> p t", p=128),
                          in_=fbuf[:])

    nc.finalize()
    return nc


def _get_nc():
    if "nc" not in _cached:
        _cached["nc"] = build_bass()
    return _cached["nc"]


def kernel(features: np.ndarray, centroids: np.ndarray) -> np.ndarray:
    features = np.ascontiguousarray(np.asarray(features, dtype=np.float32))
    centroids = np.ascontiguousarray(np.asarray(centroids, dtype=np.float32))
    # PE computes f @ cent_dev; pass 2*c so PSUM holds 2*cross directly
    # (power-of-2 scaling is exact and commutes with fp32 rounding).
    cent2 = (2.0 * centroids).astype(np.float32)
    ncsq = -(centroids.astype(np.float64) ** 2).sum(0, keepdims=True).astype(np.float32)

    nc = _get_nc()
    in_maps = [
        {
            "features": features[c * N_PER_CORE:(c + 1) * N_PER_CORE],
            "centroids": cent2,
            "ncsq": ncsq,
        }
        for c in range(N_CORES)
    ]
    res = run_bass_kernel_spmd(nc, in_maps, list(range(N_CORES))).results
    out = np.concatenate([res[c]["out"] for c in range(N_CORES)], axis=0)
    return out.astype(np.float32)


def _self_test():
    rng = np.random.default_rng(0)
    f = rng.standard_normal((N, D)).astype(np.float32)
    c = rng.standard_normal((D, K)).astype(np.float32)
    out = kernel(f, c)
    x = f @ c
    ref = (-2 * x + (c * c).sum(0)).argmin(1)
    print("mismatch:", (out[:, 0] != ref).sum(), "/", N)


if __name__ == "__main__":
    _self_test()


# revision 9
# speedup vs baseline: 83.0528x; 83.0528x over previous
"""KMeans assignment kernel for Trainium2 (8 NeuronCores, SPMD).

argmin_k ||f_n - c_k||^2  ==  argmax_k (2*f.c_k - ||c_k||^2)   (x_sq drop is
order-preserving).  Cross products run on the PE array in float32r
(12-mantissa-bit fp32) with a 3-pass hi/lo split:
    f@c ~= hi_f@hi_c + hi_f@lo_c + lo_f@hi_c        (abs err ~2e-5, fp32-grade)
at 1 cycle/row instead of fp32's 4.  The -|c|^2 bias enters as two rank-1
f32r matmuls into PSUM (or a DVE tensor-tensor add, see bias_on_dve).
Row-wise argmax via DVE max/max_index.

Sharding: features split over N across 8 cores (data parallel); centroids
replicated; no cross-core communication.
"""
import sys

sys.path.insert(0, "/opt/trn_rl_repo")

import numpy as np
from contextlib import ExitStack, nullcontext

import concourse.bacc as bacc
import concourse.mybir as mybir
from concourse import tile
from concourse.bass_utils import run_bass_kernel_spmd
from concourse.masks import make_identity

N, D, K = 131072, 512, 1024
N_CORES = 8
N_PER_CORE = N // N_CORES          # 16384
N_TILES = N_PER_CORE // 128        # 128 row-tiles per core
ND = D // 128                      # 4 contraction tiles
F32 = mybir.dt.float32
F32R = mybir.dt.float32r
U32 = mybir.dt.uint32

_cached = {}


def build_bass(n_tiles: int = N_TILES, repeat: int = 1,
               bias_on_dve: bool = False, hilo_engine: str = "vector"):
    n_rows = n_tiles * 128
    nc = bacc.Bacc()
    feat = nc.declare_dram_parameter("features", [n_rows, D], F32, isOutput=False)
    cent = nc.declare_dram_parameter("centroids", [D, K], F32, isOutput=False)
    ncsq = nc.declare_dram_parameter("ncsq", [1, K], F32, isOutput=False)
    out = nc.declare_dram_parameter("out", [n_rows, 1], F32, isOutput=True)

    with tile.TileContext(nc) as tc, ExitStack() as ctx:
        const = ctx.enter_context(tc.tile_pool(name="const", bufs=1))
        work = ctx.enter_context(tc.tile_pool(name="work", bufs=3))
        red = ctx.enter_context(tc.tile_pool(name="red", bufs=4))
        psA = ctx.enter_context(tc.tile_pool(name="psA", bufs=2, space="PSUM"))
        psB = ctx.enter_context(tc.tile_pool(name="psB", bufs=2, space="PSUM"))

        ident = const.tile([128, 128], F32)
        make_identity(nc, ident[:])

        # centroids resident in SBUF, split hi/lo f32r; layout [128, ND*K]
        ctile = const.tile([128, ND * K], F32)
        nc.sync.dma_start(
            out=ctile[:].rearrange("p (a k) -> p a k", a=ND),
            in_=cent[:].rearrange("(a p) k -> p a k", p=128),
        )
        c_hi = const.tile([128, ND * K], F32R)
        c_lo = const.tile([128, ND * K], F32R)
        nc.vector.tensor_copy(out=c_hi[:], in_=ctile[:])
        nc.vector.tensor_tensor(out=c_lo[:], in0=ctile[:], in1=c_hi[:].bitcast(F32),
                                op=mybir.AluOpType.subtract)

        # -|c|^2 bias row, split hi/lo; plus a ones row for rank-1 matmuls
        ncsq_t = const.tile([1, K], F32)
        nc.sync.dma_start(out=ncsq_t[:], in_=ncsq[:])
        if bias_on_dve:
            ncsq_b = const.tile([128, K], F32)
            nc.gpsimd.partition_broadcast(ncsq_b[:], ncsq_t[:])
        else:
            ncsq_hi = const.tile([1, K], F32R)
            ncsq_lo = const.tile([1, K], F32R)
            nc.vector.tensor_copy(out=ncsq_hi[:], in_=ncsq_t[:])
            nc.vector.tensor_tensor(out=ncsq_lo[:], in0=ncsq_t[:],
                                    in1=ncsq_hi[:].bitcast(F32),
                                    op=mybir.AluOpType.subtract)
            ones_f = const.tile([1, 128], F32)
            nc.vector.memset(ones_f[:], 1.0)
            ones_t = const.tile([1, 128], F32R)
            nc.vector.tensor_copy(out=ones_t[:], in_=ones_f[:])

        # per-row argmax indices accumulate here ([p, t*8] layout), cast at end
        idx8 = const.tile([128, n_tiles * 8], U32)
        fbuf = const.tile([128, n_tiles], F32)

        hilo = nc.vector if hilo_engine == "vector" else nc.gpsimd

        loop_ctx = tc.For_i(0, repeat, 1) if repeat > 1 else nullcontext()
        with loop_ctx:
            for rt in range(n_tiles):
                ftile = work.tile([128, D], F32, tag="ftile")
                nc.sync.dma_start(out=ftile[:], in_=feat[rt * 128:(rt + 1) * 128, :])

                # transpose features tile -> [D, rows] chunks (exact fp32)
                tp = psA.tile([128, ND * 128], F32, tag="tp")
                for d in range(ND):
                    nc.tensor.transpose(tp[:, d * 128:(d + 1) * 128],
                                        ftile[:, d * 128:(d + 1) * 128], ident[:])
                ftT = work.tile([128, D], F32, tag="ftT")
                nc.scalar.copy(out=ftT[:], in_=tp[:])

                # hi/lo split (round-to-nearest-12-bit + exact residual)
                f_hi = work.tile([128, D], F32R, tag="f_hi")
                f_lo = work.tile([128, D], F32R, tag="f_lo")
                hilo.tensor_copy(out=f_hi[:], in_=ftT[:])
                hilo.tensor_tensor(out=f_lo[:], in0=ftT[:],
                                   in1=f_hi[:].bitcast(F32),
                                   op=mybir.AluOpType.subtract)

                # m = 2*cross [- |c|^2] accumulated in PSUM [128, K]
                mp = psB.tile([128, K], F32, tag="mp")
                for kh in range(2):
                    ks = slice(kh * 512, (kh + 1) * 512)
                    mslc = mp[:, ks]
                    first = True
                    for fa, ca in ((f_hi, c_hi), (f_hi, c_lo), (f_lo, c_hi)):
                        for d in range(ND):
                            nc.tensor.matmul(
                                mslc,
                                lhsT=fa[:, d * 128:(d + 1) * 128],
                                rhs=ca[:, d * K + kh * 512:d * K + (kh + 1) * 512],
                                start=first,
                                stop=bias_on_dve and fa is f_lo and d == ND - 1)
                            first = False
                    if not bias_on_dve:
                        nc.tensor.matmul(mslc, lhsT=ones_t[:], rhs=ncsq_hi[:, ks],
                                         start=False, stop=False)
                        nc.tensor.matmul(mslc, lhsT=ones_t[:], rhs=ncsq_lo[:, ks],
                                         start=False, stop=True)

                mv = red.tile([128, 8], F32, tag="mv")
                if bias_on_dve:
                    m_s = work.tile([128, K], F32, tag="m_s")
                    nc.vector.tensor_tensor(out=m_s[:], in0=mp[:], in1=ncsq_b[:],
                                            op=mybir.AluOpType.add)
                    nc.vector.max(mv[:], m_s[:])
                    nc.vector.max_index(idx8[:, rt * 8:(rt + 1) * 8], mv[:], m_s[:])
                else:
                    nc.vector.max(mv[:], mp[:])
                    nc.vector.max_index(idx8[:, rt * 8:(rt + 1) * 8], mv[:], mp[:])

        # gather col 0 of each 8-block, cast u32 -> f32, store
        nc.vector.tensor_copy(out=fbuf[:], in_=idx8[:, 0:n_tiles * 8:8])
        nc.sync.dma_start(out=out[:, 0].rearrange("(t p) -> p t", p=128),
                          in_=fbuf[:])

    nc.finalize()
    return nc


def _get_nc():
    if "nc" not in _cached:
        _cached["nc"] = build_bass(bias_on_dve=True)
    return _cached["nc"]


def kernel(features: np.ndarray, centroids: np.ndarray) -> np.ndarray:
    features = np.ascontiguousarray(np.asarray(features, dtype=np.float32))
    centroids = np.ascontiguousarray(np.asarray(centroids, dtype=np.float32))
    # PE computes f @ cent_dev; pass 2*c so PSUM holds 2*cross directly
    # (power-of-2 scaling is exact and commutes with fp32 rounding).
    cent2 = (2.0 * centroids).astype(np.float32)
    ncsq = -(centroids.astype(np.float64) ** 2).sum(0, keepdims=True).astype(np.float32)

    nc = _get_nc()
    in_maps = [
        {
            "features": features[c * N_PER_CORE:(c + 1) * N_PER_CORE],
            "centroids": cent2,
            "ncsq": ncsq,
        }
        for c in range(N_CORES)
    ]
    res = run_bass_kernel_spmd(nc, in_maps, list(range(N_CORES))).results
    out = np.concatenate([res[c]["out"] for c in range(N_CORES)], axis=0)
    return out.astype(np.float32)


def _self_test():
    rng = np.random.default_rng(0)
    f = rng.standard_normal((N, D)).astype(np.float32)
    c = rng.standard_normal((D, K)).astype(np.float32)
    out = kernel(f, c)
    x = f @ c
    ref = (-2 * x + (c * c).sum(0)).argmin(1)
    print("mismatch:", (out[:, 0] != ref).sum(), "/", N)


if __name__ == "__main__":
    _self_test()


# revision 36
# speedup vs baseline: 85.3241x; 1.0273x over previous
"""KMeans assignment kernel for Trainium2 (8 NeuronCores, SPMD).

argmin_k ||f_n - c_k||^2  ==  argmax_k (2*f.c_k - ||c_k||^2)   (x_sq drop is
order-preserving).  Cross products run on the PE array with a 3-pass hi/lo
split at 1 cycle/row (plain fp32 matmul is 4 cyc/row):
    f@c ~= hi_f@hi_c + hi_f16@lo_c16 + lo_f16@hi_c16
where hi = float32r(f) (round-to-nearest 12-bit mantissa) and the small
correction passes run in fp16 (their ~2^-12-relative terms only need ~11
bits).  Total abs err ~2e-4 on the 2*cross scale — fp32-grade, 0 argmin flips
vs the fp32 reference.  The -|c|^2 bias is added by a DVE tensor-tensor op
(PSUM + broadcast row), then row-wise argmax via DVE max/max_index.

Sharding: features split over N across 8 cores (data parallel); centroids
replicated; no cross-core communication.
"""
import sys

sys.path.insert(0, "/opt/trn_rl_repo")

import numpy as np
from contextlib import ExitStack, nullcontext

import concourse.bacc as bacc
import concourse.mybir as mybir
from concourse import tile
from concourse.bass_utils import run_bass_kernel_spmd
from concourse.masks import make_identity

N, D, K = 131072, 512, 1024
N_CORES = 8
N_PER_CORE = N // N_CORES          # 16384
N_TILES = N_PER_CORE // 128        # 128 row-tiles per core
ND = D // 128                      # 4 contraction tiles
F32 = mybir.dt.float32
F32R = mybir.dt.float32r
F16 = mybir.dt.float16
U32 = mybir.dt.uint32

_cached = {}

# shipped configuration (see build_bass options)
SHIP_KW = {"bias_on_dve": True, "corr_f16": True}


def build_bass(n_tiles: int = N_TILES, repeat: int = 1,
               bias_on_dve: bool = False, hilo_engine: str = "vector",
               n_passes: int = 3, do_argmax: bool = True,
               corr_f16: bool = False, all_f16: bool = False,
               kh_inner: bool = False, psum_bufs: int = 2):
    n_rows = n_tiles * 128
    nc = bacc.Bacc()
    feat = nc.declare_dram_parameter("features", [n_rows, D], F32, isOutput=False)
    cent = nc.declare_dram_parameter("centroids", [D, K], F32, isOutput=False)
    ncsq = nc.declare_dram_parameter("ncsq", [1, K], F32, isOutput=False)
    out = nc.declare_dram_parameter("out", [n_rows, 1], F32, isOutput=True)

    with tile.TileContext(nc) as tc, ExitStack() as ctx:
        const = ctx.enter_context(tc.tile_pool(name="const", bufs=1))
        work = ctx.enter_context(tc.tile_pool(name="work", bufs=3))
        red = ctx.enter_context(tc.tile_pool(name="red", bufs=4))
        psA = ctx.enter_context(tc.tile_pool(name="psA", bufs=2, space="PSUM"))
        psB = ctx.enter_context(tc.tile_pool(name="psB", bufs=psum_bufs, space="PSUM"))

        ident = const.tile([128, 128], F32)
        make_identity(nc, ident[:])

        # centroids resident in SBUF, split hi/lo f32r; layout [128, ND*K]
        ctile = const.tile([128, ND * K], F32)
        nc.sync.dma_start(
            out=ctile[:].rearrange("p (a k) -> p a k", a=ND),
            in_=cent[:].rearrange("(a p) k -> p a k", p=128),
        )
        if all_f16:
            # pure-fp16 3-way split: h+l capture ~21 mantissa bits
            c_hi = const.tile([128, ND * K], F16)
            c_lo = const.tile([128, ND * K], F16)
            nc.vector.tensor_copy(out=c_hi[:], in_=ctile[:])
            nc.vector.tensor_tensor(out=c_lo[:], in0=ctile[:], in1=c_hi[:],
                                    op=mybir.AluOpType.subtract)
        else:
            c_hi = const.tile([128, ND * K], F32R)
            c_lo = const.tile([128, ND * K], F32R)
            nc.vector.tensor_copy(out=c_hi[:], in_=ctile[:])
            nc.vector.tensor_tensor(out=c_lo[:], in0=ctile[:], in1=c_hi[:].bitcast(F32),
                                    op=mybir.AluOpType.subtract)
        if corr_f16:
            # correction operands in fp16: 2-byte weight loads, ample precision
            # (error ~2^-11 relative of a ~2^-12-relative correction term)
            c_hi16 = const.tile([128, ND * K], F16)
            c_lo16 = const.tile([128, ND * K], F16)
            nc.vector.tensor_copy(out=c_hi16[:], in_=ctile[:])
            nc.vector.tensor_copy(out=c_lo16[:], in_=c_lo[:].bitcast(F32))

        # -|c|^2 bias row, split hi/lo; plus a ones row for rank-1 matmuls
        ncsq_t = const.tile([1, K], F32)
        nc.sync.dma_start(out=ncsq_t[:], in_=ncsq[:])
        if bias_on_dve:
            ncsq_b = const.tile([128, K], F32)
            nc.gpsimd.partition_broadcast(ncsq_b[:], ncsq_t[:])
        else:
            ncsq_hi = const.tile([1, K], F32R)
            ncsq_lo = const.tile([1, K], F32R)
            nc.vector.tensor_copy(out=ncsq_hi[:], in_=ncsq_t[:])
            nc.vector.tensor_tensor(out=ncsq_lo[:], in0=ncsq_t[:],
                                    in1=ncsq_hi[:].bitcast(F32),
                                    op=mybir.AluOpType.subtract)
            ones_f = const.tile([1, 128], F32)
            nc.vector.memset(ones_f[:], 1.0)
            ones_t = const.tile([1, 128], F32R)
            nc.vector.tensor_copy(out=ones_t[:], in_=ones_f[:])

        # per-row argmax indices accumulate here ([p, t*8] layout), cast at end
        idx8 = None
        if do_argmax:
            idx8 = const.tile([128, n_tiles * 8], U32, tag="idx8")
        fbuf = const.tile([128, n_tiles], F32)

        hilo = {"vector": nc.vector, "gpsimd": nc.gpsimd}.get(hilo_engine)

        loop_ctx = tc.For_i(0, repeat, 1) if repeat > 1 else nullcontext()
        with loop_ctx:
            for rt in range(n_tiles):
                ftile = work.tile([128, D], F32, tag="ftile")
                nc.sync.dma_start(out=ftile[:], in_=feat[rt * 128:(rt + 1) * 128, :])

                # transpose features tile -> [D, rows] chunks (exact fp32)
                tp = psA.tile([128, ND * 128], F32, tag="tp")
                for d in range(ND):
                    nc.tensor.transpose(tp[:, d * 128:(d + 1) * 128],
                                        ftile[:, d * 128:(d + 1) * 128], ident[:])
                ftT = work.tile([128, D], F32, tag="ftT")
                nc.scalar.copy(out=ftT[:], in_=tp[:])

                # hi/lo split + per-pass operand prep
                if all_f16:
                    f_hi = work.tile([128, D], F16, tag="f_hi")
                    f_lo = work.tile([128, D], F16, tag="f_lo")
                    nc.scalar.copy(out=f_hi[:], in_=ftT[:])
                    nc.vector.tensor_tensor(out=f_lo[:], in0=ftT[:], in1=f_hi[:],
                                            op=mybir.AluOpType.subtract)
                    passes_all = ((f_hi, c_hi), (f_hi, c_lo), (f_lo, c_hi))
                elif corr_f16:
                    f_hi = work.tile([128, D], F32R, tag="f_hi")
                    nc.vector.tensor_copy(out=f_hi[:], in_=ftT[:])
                    f_hi16 = work.tile([128, D], F16, tag="f_hi16")
                    f_lo16 = work.tile([128, D], F16, tag="f_lo16")
                    nc.scalar.copy(out=f_hi16[:], in_=ftT[:])
                    nc.vector.tensor_tensor(out=f_lo16[:], in0=ftT[:],
                                            in1=f_hi[:].bitcast(F32),
                                            op=mybir.AluOpType.subtract)
                    passes_all = ((f_hi, c_hi), (f_hi16, c_lo16), (f_lo16, c_hi16))
                else:
                    f_hi = work.tile([128, D], F32R, tag="f_hi")
                    f_lo = work.tile([128, D], F32R, tag="f_lo")
                    if hilo is None:  # "split": hi on ScalarE, lo on GpSimd
                        nc.scalar.copy(out=f_hi[:], in_=ftT[:])
                        nc.gpsimd.tensor_tensor(out=f_lo[:], in0=ftT[:],
                                                in1=f_hi[:].bitcast(F32),
                                                op=mybir.AluOpType.subtract)
                    else:
                        hilo.tensor_copy(out=f_hi[:], in_=ftT[:])
                        hilo.tensor_tensor(out=f_lo[:], in0=ftT[:],
                                           in1=f_hi[:].bitcast(F32),
                                           op=mybir.AluOpType.subtract)
                    passes_all = ((f_hi, c_hi), (f_hi, c_lo), (f_lo, c_hi))

                # m = 2*cross [- |c|^2] accumulated in PSUM [128, K]
                mp = psB.tile([128, K], F32, tag="mp")
                passes = passes_all[:n_passes]
                if kh_inner:
                    # consecutive MM pairs share the stationary operand and
                    # alternate PSUM banks
                    for pi, (fa, ca) in enumerate(passes):
                        for d in range(ND):
                            is_last_main = pi == n_passes - 1 and d == ND - 1
                            for kh in range(2):
                                nc.tensor.matmul(
                                    mp[:, kh * 512:(kh + 1) * 512],
                                    lhsT=fa[:, d * 128:(d + 1) * 128],
                                    rhs=ca[:, d * K + kh * 512:d * K + (kh + 1) * 512],
                                    start=pi == 0 and d == 0,
                                    stop=bias_on_dve and is_last_main)
                else:
                    for kh in range(2):
                        ks = slice(kh * 512, (kh + 1) * 512)
                        mslc = mp[:, ks]
                        first = True
                        for pi, (fa, ca) in enumerate(passes):
                            for d in range(ND):
                                is_last_main = pi == n_passes - 1 and d == ND - 1
                                nc.tensor.matmul(
                                    mslc,
                                    lhsT=fa[:, d * 128:(d + 1) * 128],
                                    rhs=ca[:, d * K + kh * 512:d * K + (kh + 1) * 512],
                                    start=first,
                                    stop=bias_on_dve and is_last_main)
                                first = False
                if not bias_on_dve:
                    for kh in range(2):
                        ks = slice(kh * 512, (kh + 1) * 512)
                        nc.tensor.matmul(mp[:, ks], lhsT=ones_t[:], rhs=ncsq_hi[:, ks],
                                         start=False, stop=False)
                        nc.tensor.matmul(mp[:, ks], lhsT=ones_t[:], rhs=ncsq_lo[:, ks],
                                         start=False, stop=True)

                if not do_argmax:
                    continue
                mv = red.tile([128, 8], F32, tag="mv")
                if bias_on_dve:
                    m_s = work.tile([128, K], F32, tag="m_s")
                    nc.vector.tensor_tensor(out=m_s[:], in0=mp[:], in1=ncsq_b[:],
                                            op=mybir.AluOpType.add)
                    nc.vector.max(mv[:], m_s[:])
                    nc.vector.max_index(idx8[:, rt * 8:(rt + 1) * 8], mv[:], m_s[:])
                else:
                    nc.vector.max(mv[:], mp[:])
                    nc.vector.max_index(idx8[:, rt * 8:(rt + 1) * 8], mv[:], mp[:])

        # gather col 0 of each 8-block, cast u32 -> f32, store
        if do_argmax:
            nc.vector.tensor_copy(out=fbuf[:], in_=idx8[:, 0:n_tiles * 8:8])
        else:
            nc.vector.memset(fbuf[:], 0.0)
        nc.sync.dma_start(out=out[:, 0].rearrange("(t p) -> p t", p=128),
                          in_=fbuf[:])

    nc.finalize()
    return nc


def _get_nc():
    if "nc" not in _cached:
        _cached["nc"] = build_bass(**SHIP_KW)
    return _cached["nc"]


def kernel(features: np.ndarray, centroids: np.ndarray) -> np.ndarray:
    features = np.ascontiguousarray(np.asarray(features, dtype=np.float32))
    centroids = np.ascontiguousarray(np.asarray(centroids, dtype=np.float32))
    # PE computes f @ cent_dev; pass 2*c so PSUM holds 2*cross directly
    # (power-of-2 scaling is exact and commutes with fp32 rounding).
    cent2 = (2.0 * centroids).astype(np.float32)
    ncsq = -(centroids.astype(np.float64) ** 2).sum(0, keepdims=True).astype(np.float32)

    nc = _get_nc()
    in_maps = [
        {
            "features": features[c * N_PER_CORE:(c + 1) * N_PER_CORE],
            "centroids": cent2,
            "ncsq": ncsq,
        }
        for c in range(N_CORES)
    ]
    res = run_bass_kernel_spmd(nc, in_maps, list(range(N_CORES))).results
    out = np.concatenate([res[c]["out"] for c in range(N_CORES)], axis=0)
    return out.astype(np.float32)


def _self_test():
    rng = np.random.default_rng(0)
    f = rng.standard_normal((N, D)).astype(np.float32)
    c = rng.standard_normal((D, K)).astype(np.float32)
    out = kernel(f, c)
    x = f @ c
    ref = (-2 * x + (c * c).sum(0)).argmin(1)
    print("mismatch:", (out[:, 0] != ref).sum(), "/", N)


if __name__ == "__main__":
    _self_test()
